# revision 46
# baseline (speedup 1.0000x reference)
"""Trainium2 Bass kernel for 16-head causal MultiHeadAttention.

Problem: B=2, S=2048, D=1024, H=16 (head_dim 64), causal mask, f32 I/O.

Sharding (8 cores): core c handles batch b = c//4 and head-block hb = c%4
(4 heads = 256 embedding channels). Q/K/V projections are tensor-parallel
column slices; the output projection is tensor-parallel over rows of Wo.T,
so each core emits a partial (S, D) output that the host sums per batch.

Per-core kernel (all matmuls bf16, f32 accumulate):
  1. QT/KT = (Wq_c.T).T @ xT (+bias)   -> (256, 2048) SBUF, e in partitions
  2. V    = xT.T @ Wv_c.T              -> (2048, 256) "V_aug" layout with a
     ones column per head (for softmax row sums via the same matmul)
  3. scoresT[k, q] = KT.T @ QT per 128-key block (both heads of a pair per
     pass), causal blocks only; the triangular -1e9 bias on diagonal
     blocks is accumulated into PSUM with an identity-matmul (keeps the
     DVE out of the scores->exp->ctx critical chain); exp on ScalarE
     (scale=1/8, no max subtraction -- scores are O(5) so exp is safe)
  4. ctxT' (and row sums) = V_aug.T @ expT accumulated over key blocks
  5. normalize: reciprocal of row sums, broadcast across partitions with a
     K=1 matmul, multiply on VectorE -> ctxT bf16
  6. out_partial = ctxT.T @ Wo_c.T    -> (2048, 1024) bf16 DMA'd out

Emission interleaves projection/output-projection work units into the
exp-bound attention pipeline steps so the PE always has filler work.

Host: out[b] = sum of the 4 partials + (Wo @ bv + bo).
"""

import sys

for _p in ("/root/.axon_site/_ro/trn_rl_repo", "/opt/trn_rl_repo"):
    if _p not in sys.path:
        sys.path.append(_p)

from collections import deque
from functools import partial

import numpy as np
import ml_dtypes

import concourse.mybir as mybir
import concourse.tile as tile
from concourse import bacc, bass_utils

B, S, D, H = 2, 2048, 1024, 16
HD = D // H  # 64
NCORES = 8
EPC = 256  # embedding channels per core (4 heads)
BF16 = mybir.dt.bfloat16
F32 = mybir.dt.float32
NEG = -1.0e9

_compiled_cache: dict[str, "bacc.Bacc"] = {}


def _kbs_for(qt: int, mode: str):
    """[(kb, q_lo)] for one 512-wide query tile."""
    if mode == "causal":
        return [(kb, 128 * (kb - 4 * qt) if kb >= 4 * qt else 0)
                for kb in range(4 * qt + 4)]
    return [(kb, 0) for kb in range(S // 128)]


def build_nc(mode: str = "causal") -> "bacc.Bacc":
    nc = bacc.Bacc("TRN2")

    xq = nc.dram_tensor("xq_t", (D, S), BF16, kind="ExternalInput")
    xk = nc.dram_tensor("xk_t", (D, S), BF16, kind="ExternalInput")
    xv = nc.dram_tensor("xv_t", (D, S), BF16, kind="ExternalInput")
    wq = nc.dram_tensor("wq_t", (D, EPC), BF16, kind="ExternalInput")
    wk = nc.dram_tensor("wk_t", (D, EPC), BF16, kind="ExternalInput")
    wv = nc.dram_tensor("wv_t", (D, EPC), BF16, kind="ExternalInput")
    wo = nc.dram_tensor("wo_t", (EPC, D), BF16, kind="ExternalInput")
    bq = nc.dram_tensor("bq2", (2, 128, 1), F32, kind="ExternalInput")
    bk = nc.dram_tensor("bk2", (2, 128, 1), F32, kind="ExternalInput")
    tri = nc.dram_tensor("tri", (128, 128), BF16, kind="ExternalInput")
    ones = nc.dram_tensor("ones", (33, 128), mybir.dt.float32r,
                          kind="ExternalInput")
    eye = nc.dram_tensor("eye", (128, 128), BF16, kind="ExternalInput")
    out = nc.dram_tensor("out", (S, D), BF16, kind="ExternalOutput")

    n_cb = D // 128  # 8 contraction blocks
    n_tb = S // 128  # 16 token blocks
    n_qt = S // 512  # 4 query tiles

    xq_v = xq.rearrange("(cb p) t -> p cb t", p=128)
    xk_v = xk.rearrange("(cb p) t -> p cb t", p=128)
    xv_v = xv.rearrange("(cb p) t -> p cb t", p=128)
    wq_v = wq.rearrange("(cb p) e -> p cb e", p=128)
    wk_v = wk.rearrange("(cb p) e -> p cb e", p=128)
    wv_v = wv.rearrange("(cb p) e -> p cb e", p=128)
    wo_v = wo.rearrange("(eb p) o -> p eb o", p=128)

    with tile.TileContext(nc) as tc:
        with (
            tc.tile_pool(name="consts", bufs=1) as consts,
            tc.tile_pool(name="qkt", bufs=1) as qkt_pool,
            tc.tile_pool(name="vaug", bufs=1) as vaug_pool,
            tc.tile_pool(name="ctxt", bufs=1) as ctxt_pool,
            tc.tile_pool(name="attn_sb", bufs=8) as attn_sb,
            tc.tile_pool(name="norm_sb", bufs=8) as norm_sb,
            tc.tile_pool(name="out_sb", bufs=4) as out_sb,
            # PSUM: 8 banks total = psS 2x2 + psC 3x1 (pc+pb) + psMM 1x1
            tc.tile_pool(name="psS", bufs=2, space="PSUM") as psS,
            tc.tile_pool(name="psC", bufs=2, space="PSUM") as psC,
            tc.tile_pool(name="psMM", bufs=2, space="PSUM") as psMM,
        ):
            # --- resident SBUF tensors ---
            xq_sb = consts.tile([128, n_cb * S], BF16, name="xq_sb")
            xk_sb = consts.tile([128, n_cb * S], BF16, name="xk_sb")
            xv_sb = consts.tile([128, n_cb * S], BF16, name="xv_sb")
            wq_sb = consts.tile([128, n_cb * EPC], BF16, name="wq_sb")
            wk_sb = consts.tile([128, n_cb * EPC], BF16, name="wk_sb")
            wv_sb = consts.tile([128, n_cb * EPC], BF16, name="wv_sb")
            wo_sb = consts.tile([128, 2 * D], BF16, name="wo_sb")
            tri_sb = consts.tile([128, 128], BF16, name="tri_sb")
            eye_sb = consts.tile([128, 128], BF16, name="eye_sb")
            bq_sb = consts.tile([128, 2], F32, name="bq_sb")
            bk_sb = consts.tile([128, 2], F32, name="bk_sb")
            ones_row = consts.tile([33, 128], mybir.dt.float32r, name="ones_row")

            qt_sb = [qkt_pool.tile([128, S], BF16, name=f"qt_sb{eb}", tag=f"qt{eb}")
                     for eb in range(2)]
            kt_sb = [qkt_pool.tile([128, S], BF16, name=f"kt_sb{eb}", tag=f"kt{eb}")
                     for eb in range(2)]
            vaug = [vaug_pool.tile([128, 4 * 65], BF16, name=f"vaug{tb}")
                    for tb in range(n_tb)]
            ctxt_sb = [ctxt_pool.tile([128, S], BF16, name=f"ctxt_sb{eb}")
                       for eb in range(2)]

            def xs(t, cb):
                return t[:, cb * S:(cb + 1) * S]

            def ws(t, cb):
                return t[:, cb * EPC:(cb + 1) * EPC]

            def dma_chunk(ts_):
                """One wide DMA per tensor for the ts_ column chunk, in
                consumption order K, Q, V."""
                cs = slice(ts_ * 512, (ts_ + 1) * 512)
                for xsb, dview in ((xk_sb, xk_v), (xq_sb, xq_v), (xv_sb, xv_v)):
                    nc.sync.dma_start(
                        xsb[:].rearrange("p (cb t) -> p cb t", cb=n_cb)[:, :, cs],
                        dview[:, :, cs],
                    )

            def qk_unit(xsb, wsb, bias, dst, eb, ts_):
                cs = slice(ts_ * 512, (ts_ + 1) * 512)
                ps = psMM.tile([128, 512], F32, name="proj_ps", tag="mm")
                for cb in range(n_cb):
                    nc.tensor.matmul(
                        ps[:],
                        lhsT=ws(wsb, cb)[:, eb * 128:(eb + 1) * 128],
                        rhs=xs(xsb, cb)[:, cs],
                        start=(cb == 0), stop=(cb == n_cb - 1),
                    )
                nc.vector.tensor_scalar_add(dst[eb][:, cs], ps[:],
                                            bias[:, eb:eb + 1])

            def v_unit(tb):
                ps = psMM.tile([128, EPC], F32, name="v_ps", tag="mm")
                for cb in range(n_cb):
                    nc.tensor.matmul(
                        ps[:],
                        lhsT=xs(xv_sb, cb)[:, tb * 128:(tb + 1) * 128],
                        rhs=ws(wv_sb, cb),
                        start=(cb == 0), stop=(cb == n_cb - 1),
                    )
                va = vaug[tb].rearrange("p (h x) -> p h x", h=4)
                nc.vector.memset(va[:, :, 64:65], 1.0)
                nc.vector.tensor_copy(va[:, :, 0:64],
                                      ps.rearrange("p (h d) -> p h d", h=4))

            def proj_units(ts_):
                units = []
                for xsb, wsb, bias, dst in (
                    (xk_sb, wk_sb, bk_sb, kt_sb),
                    (xq_sb, wq_sb, bq_sb, qt_sb),
                ):
                    for eb in range(2):
                        units.append(partial(qk_unit, xsb, wsb, bias, dst,
                                             eb, ts_))
                for tb in range(4 * ts_, 4 * ts_ + 4):
                    units.append(partial(v_unit, tb))
                return units

            def outproj_unit(tb):
                ot = out_sb.tile([128, D], BF16, name="out_t", tag="ot")
                for nb in range(2):
                    po = psMM.tile([128, 512], F32, name="out_ps", tag="mm")
                    for eb in range(2):
                        nc.tensor.matmul(
                            po[:],
                            lhsT=ctxt_sb[eb][:, tb * 128:(tb + 1) * 128],
                            rhs=wo_sb[:, eb * D + nb * 512:
                                      eb * D + (nb + 1) * 512],
                            start=(eb == 0), stop=(eb == 1),
                        )
                    nc.vector.tensor_copy(ot[:, nb * 512:(nb + 1) * 512],
                                          po[:])
                nc.sync.dma_start(out[tb * 128:(tb + 1) * 128, :], ot[:])

            proj_q = deque()
            out_q = deque()
            norm_q = deque()
            allow_out = [False]

            def pop_filler(n=1):
                for _ in range(n):
                    if norm_q:
                        norm_q.popleft()()
                    elif proj_q:
                        proj_q.popleft()()
                    elif out_q and allow_out[0]:
                        out_q.popleft()()

            def attention(qt):
                for hp in range(2):  # head pair (heads 2hp, 2hp+1)
                    kbs = _kbs_for(qt, mode)
                    pc0 = psC.tile([65, 512], F32, name="pc0", tag="pc")
                    pc1 = psC.tile([65, 512], F32, name="pc1", tag="pc")
                    ets = {}
                    # software pipeline: scores/exp for step i, ctx for i-2
                    LAG = 4
                    for i in range(len(kbs) + LAG):
                        if i < len(kbs):
                            kb, q_lo = kbs[i]
                            w = 512 - q_lo
                            crossing = mode == "causal" and kb >= 4 * qt
                            ps = psS.tile([128, 1024], F32, name="sc_ps",
                                          tag="sc")
                            qs = qt * 512 + q_lo
                            for h2 in range(2):
                                nc.tensor.matmul(
                                    ps[:, 512 * h2 + q_lo:512 * h2 + 512],
                                    lhsT=kt_sb[hp][64 * h2:64 * h2 + 64,
                                                   kb * 128:(kb + 1) * 128],
                                    rhs=qt_sb[hp][64 * h2:64 * h2 + 64,
                                                  qs:qs + w],
                                    start=True, stop=not crossing,
                                )
                            if crossing:
                                # accumulate the triangular -1e9 bias into
                                # the diagonal 128 columns via identity mm
                                for h2 in range(2):
                                    o = 512 * h2 + q_lo
                                    nc.tensor.matmul(
                                        ps[:, o:o + 128],
                                        lhsT=eye_sb[:], rhs=tri_sb[:],
                                        start=False, stop=True,
                                    )
                            et = attn_sb.tile([128, 1024], BF16, name="exp_t",
                                              tag="exp")
                            psg = ps.rearrange("p (g c) -> p g c", g=2)
                            etg = et.rearrange("p (g c) -> p g c", g=2)
                            nc.scalar.activation(
                                etg[:, :, q_lo:512], psg[:, :, q_lo:512],
                                mybir.ActivationFunctionType.Exp,
                                scale=0.125,
                            )
                            ets[i] = et
                        if i >= LAG:
                            kb, q_lo = kbs[i - LAG]
                            et = ets.pop(i - LAG)
                            first = (i - LAG == 0)
                            last = (i == len(kbs) + LAG - 1)
                            for h2, pc in ((0, pc0), (1, pc1)):
                                hh = 2 * hp + h2
                                nc.tensor.matmul(
                                    pc[:, q_lo:512],
                                    lhsT=vaug[kb][:, 65 * hh:65 * hh + 65],
                                    rhs=et[:, 512 * h2 + q_lo:512 * h2 + 512],
                                    start=first, stop=last,
                                )
                        pop_filler()
                    # normalize both heads; DVE reads free the pc slots
                    rec = norm_sb.tile([33, 512], mybir.dt.float32r, name="rec",
                                       tag="rec")
                    ctmps = []
                    for h2, pc in ((0, pc0), (1, pc1)):
                        ctmp = norm_sb.tile([64, 512], BF16, name="ctmp",
                                            tag=f"ctmp{h2}")
                        nc.vector.tensor_copy(ctmp[:], pc[0:64, :])
                        ctmps.append(ctmp)
                        with nc.allow_low_precision(
                                reason="softmax 1/rowsum in f32r"):
                            nc.vector.reciprocal(rec[32 * h2:32 * h2 + 1, :],
                                                 pc[64:65, :])
                    pb = psC.tile([128, 512], F32, name="pb", tag="pc")
                    nc.tensor.matmul(pb[:], lhsT=ones_row[:], rhs=rec[:])
                    for h2 in range(2):
                        nc.vector.tensor_mul(
                            ctxt_sb[hp][64 * h2:64 * h2 + 64,
                                        qt * 512:(qt + 1) * 512],
                            ctmps[h2][:], pb[64 * h2:64 * h2 + 64, :],
                        )
                        pop_filler()

            # --- emission ---
            cs0 = slice(0, 512)
            for wsb, wview, xsb, xview in (
                (wk_sb, wk_v, xk_sb, xk_v),
                (wq_sb, wq_v, xq_sb, xq_v),
                (wv_sb, wv_v, xv_sb, xv_v),
            ):
                nc.sync.dma_start(
                    wsb[:].rearrange("p (cb e) -> p cb e", cb=n_cb), wview[:])
                nc.sync.dma_start(
                    xsb[:].rearrange("p (cb t) -> p cb t", cb=n_cb)[:, :, cs0],
                    xview[:, :, cs0],
                )
            nc.sync.dma_start(tri_sb[:], tri[:])
            nc.sync.dma_start(eye_sb[:], eye[:])
            nc.sync.dma_start(bq_sb[:].rearrange("p (eb x) -> p eb x", eb=2),
                              bq.rearrange("eb p x -> p eb x"))
            nc.sync.dma_start(bk_sb[:].rearrange("p (eb x) -> p eb x", eb=2),
                              bk.rearrange("eb p x -> p eb x"))
            nc.sync.dma_start(ones_row[:], ones[:])

            # emit K/Q projections of group 0 inline; V units go on the
            # filler queue (delivered during the first attention steps) so
            # the PE stream is not blocked waiting for the xv chunk DMA
            g0 = proj_units(0)
            for u in g0[:4]:
                u()
            proj_q.extend(g0[4:])
            qt_order = [0, 1, 2, 3]
            next_group = 1

            def req_groups(qt):
                # highest projection group attention(qt) consumes
                return qt if mode == "causal" else n_qt - 1

            for wi, qt in enumerate(qt_order):
                # this window's required groups must be fully emitted BEFORE
                # the attention instructions that read them
                while next_group <= req_groups(qt):
                    dma_chunk(next_group)
                    for u in proj_units(next_group):
                        u()
                    next_group += 1
                # queue the next window's groups as pop-filler
                need = (req_groups(qt_order[wi + 1])
                        if wi + 1 < len(qt_order) else -1)
                while next_group <= need:
                    dma_chunk(next_group)
                    proj_q.extend(proj_units(next_group))
                    next_group += 1
                if wi == 0:
                    nc.sync.dma_start(
                        wo_sb[:].rearrange("p (eb o) -> p eb o", eb=2), wo_v[:])
                allow_out[0] = wi == 3
                attention(qt)
                while proj_q:  # drain before the next attention tile
                    proj_q.popleft()()
                out_q.extend(partial(outproj_unit, tb)
                             for tb in range(4 * qt, 4 * qt + 4))
            while norm_q:
                norm_q.popleft()()
            while out_q:
                out_q.popleft()()

    nc.compile()
    return nc


def get_compiled(mode: str = "causal") -> "bacc.Bacc":
    nc = _compiled_cache.get(mode)
    if nc is None:
        nc = build_nc(mode)
        _compiled_cache[mode] = nc
    return nc


def _sel_np():
    sel = np.zeros((33, 128), np.float32)
    sel[0, 0:64] = 1.0
    sel[32, 64:128] = 1.0
    return sel


def kernel(query, key, value, mask, Wq, bq, Wk, bk, Wv, bv, Wo, bo):
    query = np.asarray(query, np.float32)
    key = np.asarray(key, np.float32)
    value = np.asarray(value, np.float32)
    mask = np.asarray(mask)
    Wq, bq = np.asarray(Wq, np.float32), np.asarray(bq, np.float32)
    Wk, bk = np.asarray(Wk, np.float32), np.asarray(bk, np.float32)
    Wv, bv = np.asarray(Wv, np.float32), np.asarray(bv, np.float32)
    Wo, bo = np.asarray(Wo, np.float32), np.asarray(bo, np.float32)

    trilm = np.tril(np.ones((S, S), mask.dtype))
    if all(np.array_equal(mask[b], trilm) for b in range(B)):
        mode = "causal"
    elif mask.all():
        mode = "full"
    else:
        raise NotImplementedError("general mask not supported")

    bf = ml_dtypes.bfloat16
    xT = {}
    for nm, arr in (("q", query), ("k", key), ("v", value)):
        xT[nm] = [np.ascontiguousarray(arr[b].T).astype(bf) for b in range(B)]
    WqT = Wq.T.astype(bf)
    WkT = Wk.T.astype(bf)
    WvT = Wv.T.astype(bf)
    WoT = np.ascontiguousarray(Wo.T).astype(bf)
    tri_np = np.where(
        np.arange(128)[:, None] > np.arange(128)[None, :], NEG, 0.0
    ).astype(bf)
    eye_np = np.eye(128, dtype=bf)

    in_maps = []
    for c in range(NCORES):
        b, hb = c // 4, c % 4
        es = hb * EPC
        in_maps.append({
            "xq_t": xT["q"][b],
            "xk_t": xT["k"][b],
            "xv_t": xT["v"][b],
            "wq_t": np.ascontiguousarray(WqT[:, es:es + EPC]),
            "wk_t": np.ascontiguousarray(WkT[:, es:es + EPC]),
            "wv_t": np.ascontiguousarray(WvT[:, es:es + EPC]),
            "wo_t": np.ascontiguousarray(WoT[es:es + EPC, :]),
            "bq2": bq[es:es + EPC].reshape(2, 128, 1).astype(np.float32),
            "bk2": bk[es:es + EPC].reshape(2, 128, 1).astype(np.float32),
            "tri": tri_np,
            "eye": eye_np,
            "ones": _sel_np(),
        })

    nc = get_compiled(mode)
    res = bass_utils.run_bass_kernel_spmd(nc, in_maps, core_ids=list(range(NCORES)))

    const = Wo @ bv + bo
    outf = np.zeros((B, S, D), np.float32)
    for c in range(NCORES):
        outf[c // 4] += res.results[c]["out"].astype(np.float32)
    outf += const[None, None, :]
    return outf


# revision 56
# speedup vs baseline: 1.0416x; 1.0416x over previous
"""Trainium2 Bass kernel for 16-head causal MultiHeadAttention.

Problem: B=2, S=2048, D=1024, H=16 (head_dim 64), causal mask, f32 I/O.

Sharding (8 cores): core c handles batch b = c//4 and head-block hb = c%4
(4 heads = 256 embedding channels). Q/K/V projections are tensor-parallel
column slices; the output projection is tensor-parallel over rows of Wo.T,
so each core emits a partial (S, D) output that the host sums per batch.

Per-core kernel (all matmuls bf16, f32 accumulate):
  1. QT/KT = (Wq_c.T).T @ xT (+bias)   -> (256, 2048) SBUF, e in partitions
  2. V    = xT.T @ Wv_c.T              -> (2048, 256) "V_aug" layout with a
     ones column per head (for softmax row sums via the same matmul)
  3. scoresT[k, q] = KT.T @ QT per 128-key block (both heads of a pair per
     pass), causal blocks only; the triangular -1e9 bias on diagonal
     blocks is accumulated into PSUM with an identity-matmul (keeps the
     DVE out of the scores->exp->ctx critical chain); exp on ScalarE
     (scale=1/8, no max subtraction -- scores are O(5) so exp is safe)
  4. ctxT' (and row sums) = V_aug.T @ expT accumulated over key blocks
  5. normalize: reciprocal of row sums, broadcast across partitions with a
     K=1 matmul, multiply on VectorE -> ctxT bf16
  6. out_partial = ctxT.T @ Wo_c.T    -> (2048, 1024) bf16 DMA'd out

Emission interleaves projection/output-projection work units into the
exp-bound attention pipeline steps so the PE always has filler work.

Host: out[b] = sum of the 4 partials + (Wo @ bv + bo).
"""

import sys

for _p in ("/root/.axon_site/_ro/trn_rl_repo", "/opt/trn_rl_repo"):
    if _p not in sys.path:
        sys.path.append(_p)

from collections import deque
from functools import partial

import numpy as np
import ml_dtypes

import concourse.mybir as mybir
import concourse.tile as tile
from concourse import bacc, bass_utils

B, S, D, H = 2, 2048, 1024, 16
HD = D // H  # 64
NCORES = 8
EPC = 256  # embedding channels per core (4 heads)
BF16 = mybir.dt.bfloat16
F32 = mybir.dt.float32
NEG = -1.0e9

_compiled_cache: dict[str, "bacc.Bacc"] = {}


def _kbs_for(qt: int, mode: str):
    """[(kb, q_lo)] for one 512-wide query tile."""
    if mode == "causal":
        return [(kb, 128 * (kb - 4 * qt) if kb >= 4 * qt else 0)
                for kb in range(4 * qt + 4)]
    return [(kb, 0) for kb in range(S // 128)]


def build_nc(mode: str = "causal") -> "bacc.Bacc":
    nc = bacc.Bacc("TRN2")

    xq = nc.dram_tensor("xq_t", (D, S), BF16, kind="ExternalInput")
    xk = nc.dram_tensor("xk_t", (D, S), BF16, kind="ExternalInput")
    xv = nc.dram_tensor("xv_t", (D, S), BF16, kind="ExternalInput")
    wq = nc.dram_tensor("wq_t", (D, EPC), BF16, kind="ExternalInput")
    wk = nc.dram_tensor("wk_t", (D, EPC), BF16, kind="ExternalInput")
    wv = nc.dram_tensor("wv_t", (D, EPC), BF16, kind="ExternalInput")
    wo = nc.dram_tensor("wo_t", (EPC, D), BF16, kind="ExternalInput")
    bq = nc.dram_tensor("bq2", (2, 128, 1), F32, kind="ExternalInput")
    bk = nc.dram_tensor("bk2", (2, 128, 1), F32, kind="ExternalInput")
    tri = nc.dram_tensor("tri", (128, 128), BF16, kind="ExternalInput")
    ones = nc.dram_tensor("ones", (33, 128), mybir.dt.float32r,
                          kind="ExternalInput")
    out = nc.dram_tensor("out", (S, D), BF16, kind="ExternalOutput")

    n_cb = D // 128  # 8 contraction blocks
    n_tb = S // 128  # 16 token blocks
    n_qt = S // 512  # 4 query tiles

    xq_v = xq.rearrange("(cb p) t -> p cb t", p=128)
    xk_v = xk.rearrange("(cb p) t -> p cb t", p=128)
    xv_v = xv.rearrange("(cb p) t -> p cb t", p=128)
    wq_v = wq.rearrange("(cb p) e -> p cb e", p=128)
    wk_v = wk.rearrange("(cb p) e -> p cb e", p=128)
    wv_v = wv.rearrange("(cb p) e -> p cb e", p=128)
    wo_v = wo.rearrange("(eb p) o -> p eb o", p=128)

    with tile.TileContext(nc) as tc:
        with (
            tc.tile_pool(name="consts", bufs=1) as consts,
            tc.tile_pool(name="qkt", bufs=1) as qkt_pool,
            tc.tile_pool(name="vaug", bufs=1) as vaug_pool,
            tc.tile_pool(name="ctxt", bufs=1) as ctxt_pool,
            tc.tile_pool(name="attn_sb", bufs=8) as attn_sb,
            tc.tile_pool(name="norm_sb", bufs=8) as norm_sb,
            tc.tile_pool(name="out_sb", bufs=4) as out_sb,
            # PSUM: 8 banks total = psS 2x2 + psC 3x1 (pc+pb) + psMM 1x1
            tc.tile_pool(name="psS", bufs=2, space="PSUM") as psS,
            tc.tile_pool(name="psC", bufs=2, space="PSUM") as psC,
            tc.tile_pool(name="psMM", bufs=2, space="PSUM") as psMM,
        ):
            # --- resident SBUF tensors ---
            xq_sb = consts.tile([128, n_cb * S], BF16, name="xq_sb")
            xk_sb = consts.tile([128, n_cb * S], BF16, name="xk_sb")
            xv_sb = consts.tile([128, n_cb * S], BF16, name="xv_sb")
            wq_sb = consts.tile([128, n_cb * EPC], BF16, name="wq_sb")
            wk_sb = consts.tile([128, n_cb * EPC], BF16, name="wk_sb")
            wv_sb = consts.tile([128, n_cb * EPC], BF16, name="wv_sb")
            wo_sb = consts.tile([128, 2 * D], BF16, name="wo_sb")
            tri_sb = consts.tile([128, 128], BF16, name="tri_sb")
            bq_sb = consts.tile([128, 2], F32, name="bq_sb")
            bk_sb = consts.tile([128, 2], F32, name="bk_sb")
            ones_row = consts.tile([33, 128], mybir.dt.float32r, name="ones_row")

            qt_sb = [qkt_pool.tile([128, S], BF16, name=f"qt_sb{eb}", tag=f"qt{eb}")
                     for eb in range(2)]
            kt_sb = [qkt_pool.tile([128, S], BF16, name=f"kt_sb{eb}", tag=f"kt{eb}")
                     for eb in range(2)]
            vaug = [vaug_pool.tile([128, 4 * 65], BF16, name=f"vaug{tb}")
                    for tb in range(n_tb)]
            ctxt_sb = [ctxt_pool.tile([128, S], BF16, name=f"ctxt_sb{eb}")
                       for eb in range(2)]

            def xs(t, cb):
                return t[:, cb * S:(cb + 1) * S]

            def ws(t, cb):
                return t[:, cb * EPC:(cb + 1) * EPC]

            def dma_one(xsb, dview, ts_):
                cs = slice(ts_ * 512, (ts_ + 1) * 512)
                nc.sync.dma_start(
                    xsb[:].rearrange("p (cb t) -> p cb t", cb=n_cb)[:, :, cs],
                    dview[:, :, cs],
                )

            def dma_chunk(ts_):
                """K and Q chunks now; the V chunk is deferred into the
                filler queue so it does not contend with the scores-critical
                K/Q DMAs at window start."""
                dma_one(xk_sb, xk_v, ts_)
                dma_one(xq_sb, xq_v, ts_)

            def qk_unit(xsb, wsb, bias, dst, eb, ts_):
                cs = slice(ts_ * 512, (ts_ + 1) * 512)
                ps = psMM.tile([128, 512], F32, name="proj_ps", tag="mm")
                for cb in range(n_cb):
                    nc.tensor.matmul(
                        ps[:],
                        lhsT=ws(wsb, cb)[:, eb * 128:(eb + 1) * 128],
                        rhs=xs(xsb, cb)[:, cs],
                        start=(cb == 0), stop=(cb == n_cb - 1),
                    )
                nc.vector.tensor_scalar_add(dst[eb][:, cs], ps[:],
                                            bias[:, eb:eb + 1])

            def v_unit(tb):
                ps = psMM.tile([128, EPC], F32, name="v_ps", tag="mm")
                for cb in range(n_cb):
                    nc.tensor.matmul(
                        ps[:],
                        lhsT=xs(xv_sb, cb)[:, tb * 128:(tb + 1) * 128],
                        rhs=ws(wv_sb, cb),
                        start=(cb == 0), stop=(cb == n_cb - 1),
                    )
                va = vaug[tb].rearrange("p (h x) -> p h x", h=4)
                nc.vector.memset(va[:, :, 64:65], 1.0)
                nc.vector.tensor_copy(va[:, :, 0:64],
                                      ps.rearrange("p (h d) -> p h d", h=4))

            def proj_units(ts_):
                units = []
                for xsb, wsb, bias, dst in (
                    (xk_sb, wk_sb, bk_sb, kt_sb),
                    (xq_sb, wq_sb, bq_sb, qt_sb),
                ):
                    for eb in range(2):
                        units.append(partial(qk_unit, xsb, wsb, bias, dst,
                                             eb, ts_))
                if ts_ > 0:
                    units.append(partial(dma_one, xv_sb, xv_v, ts_))
                for tb in range(4 * ts_, 4 * ts_ + 4):
                    units.append(partial(v_unit, tb))
                return units

            def outproj_unit(tb, tail=False):
                ot = out_sb.tile([128, D], BF16, name="out_t", tag="ot")
                for nb in range(2):
                    po = psMM.tile([128, 512], F32, name="out_ps", tag="mm")
                    for eb in range(2):
                        nc.tensor.matmul(
                            po[:],
                            lhsT=ctxt_sb[eb][:, tb * 128:(tb + 1) * 128],
                            rhs=wo_sb[:, eb * D + nb * 512:
                                      eb * D + (nb + 1) * 512],
                            start=(eb == 0), stop=(eb == 1),
                        )
                    if tail and nb == 1:
                        # ScalarE is idle after the final exp; split the
                        # tail copies across both engines
                        nc.scalar.copy(ot[:, nb * 512:(nb + 1) * 512], po[:])
                    else:
                        nc.vector.tensor_copy(ot[:, nb * 512:(nb + 1) * 512],
                                              po[:])
                nc.sync.dma_start(out[tb * 128:(tb + 1) * 128, :], ot[:])

            proj_q = deque()
            out_q = deque()
            norm_q = deque()
            allow_out = [False]
            keep_back = [0]  # outproj units reserved to fill the tail

            def pop_filler(n=1):
                for _ in range(n):
                    if norm_q:
                        norm_q.popleft()()
                    elif proj_q:
                        proj_q.popleft()()
                    elif out_q and allow_out[0] and len(out_q) > keep_back[0]:
                        out_q.popleft()()

            def attention(qt):
                for hp in range(2):  # head pair (heads 2hp, 2hp+1)
                    kbs = _kbs_for(qt, mode)
                    pc0 = psC.tile([65, 512], F32, name="pc0", tag="pc")
                    pc1 = psC.tile([65, 512], F32, name="pc1", tag="pc")
                    ets = {}
                    # software pipeline: scores/exp for step i, ctx for i-2
                    LAG = 4
                    for i in range(len(kbs) + LAG):
                        if i < len(kbs):
                            kb, q_lo = kbs[i]
                            w = 512 - q_lo
                            crossing = mode == "causal" and kb >= 4 * qt
                            ps = psS.tile([128, 1024], F32, name="sc_ps",
                                          tag="sc")
                            qs = qt * 512 + q_lo
                            for h2 in range(2):
                                nc.tensor.matmul(
                                    ps[:, 512 * h2 + q_lo:512 * h2 + 512],
                                    lhsT=kt_sb[hp][64 * h2:64 * h2 + 64,
                                                   kb * 128:(kb + 1) * 128],
                                    rhs=qt_sb[hp][64 * h2:64 * h2 + 64,
                                                  qs:qs + w],
                                )
                            et = attn_sb.tile([128, 1024], BF16, name="exp_t",
                                              tag="exp")
                            psg = ps.rearrange("p (g c) -> p g c", g=2)
                            etg = et.rearrange("p (g c) -> p g c", g=2)
                            nc.scalar.activation(
                                etg[:, :, q_lo:512], psg[:, :, q_lo:512],
                                mybir.ActivationFunctionType.Exp,
                                scale=0.125,
                            )
                            if crossing:
                                # zero the masked upper half of the diagonal
                                # block with a 0/1 multiply (off the PE; the
                                # LAG-deep pipeline hides the extra DVE hop)
                                dg = etg[:, :, q_lo:q_lo + 128]
                                nc.vector.tensor_mul(
                                    dg, dg,
                                    tri_sb[:, None, :].broadcast_to(
                                        [128, 2, 128]),
                                )
                            ets[i] = et
                        if i >= LAG:
                            kb, q_lo = kbs[i - LAG]
                            et = ets.pop(i - LAG)
                            first = (i - LAG == 0)
                            last = (i == len(kbs) + LAG - 1)
                            for h2, pc in ((0, pc0), (1, pc1)):
                                hh = 2 * hp + h2
                                nc.tensor.matmul(
                                    pc[:, q_lo:512],
                                    lhsT=vaug[kb][:, 65 * hh:65 * hh + 65],
                                    rhs=et[:, 512 * h2 + q_lo:512 * h2 + 512],
                                    start=first, stop=last,
                                )
                        pop_filler()
                    # normalize both heads; DVE reads free the pc slots
                    rec = norm_sb.tile([33, 512], mybir.dt.float32r, name="rec",
                                       tag="rec")
                    ctmps = []
                    for h2, pc in ((0, pc0), (1, pc1)):
                        ctmp = norm_sb.tile([64, 512], BF16, name="ctmp",
                                            tag=f"ctmp{h2}")
                        nc.vector.tensor_copy(ctmp[:], pc[0:64, :])
                        ctmps.append(ctmp)
                        with nc.allow_low_precision(
                                reason="softmax 1/rowsum in f32r"):
                            nc.vector.reciprocal(rec[32 * h2:32 * h2 + 1, :],
                                                 pc[64:65, :])
                    pb = psC.tile([128, 512], F32, name="pb", tag="pc")
                    nc.tensor.matmul(pb[:], lhsT=ones_row[:], rhs=rec[:])
                    for h2 in range(2):
                        nc.vector.tensor_mul(
                            ctxt_sb[hp][64 * h2:64 * h2 + 64,
                                        qt * 512:(qt + 1) * 512],
                            ctmps[h2][:], pb[64 * h2:64 * h2 + 64, :],
                        )
                        pop_filler()

            # --- emission ---
            cs0 = slice(0, 512)
            for wsb, wview, xsb, xview in (
                (wk_sb, wk_v, xk_sb, xk_v),
                (wq_sb, wq_v, xq_sb, xq_v),
                (wv_sb, wv_v, xv_sb, xv_v),
            ):
                nc.sync.dma_start(
                    wsb[:].rearrange("p (cb e) -> p cb e", cb=n_cb), wview[:])
                nc.sync.dma_start(
                    xsb[:].rearrange("p (cb t) -> p cb t", cb=n_cb)[:, :, cs0],
                    xview[:, :, cs0],
                )
            nc.sync.dma_start(tri_sb[:], tri[:])
            nc.sync.dma_start(bq_sb[:].rearrange("p (eb x) -> p eb x", eb=2),
                              bq.rearrange("eb p x -> p eb x"))
            nc.sync.dma_start(bk_sb[:].rearrange("p (eb x) -> p eb x", eb=2),
                              bk.rearrange("eb p x -> p eb x"))
            nc.sync.dma_start(ones_row[:], ones[:])

            # emit K/Q projections of group 0 inline; V units go on the
            # filler queue (delivered during the first attention steps) so
            # the PE stream is not blocked waiting for the xv chunk DMA
            g0 = proj_units(0)
            for u in g0[:4]:
                u()
            proj_q.extend(g0[4:])
            qt_order = [0, 1, 2, 3]
            next_group = 1

            def req_groups(qt):
                # highest projection group attention(qt) consumes
                return qt if mode == "causal" else n_qt - 1

            for wi, qt in enumerate(qt_order):
                # this window's required groups must be fully emitted BEFORE
                # the attention instructions that read them
                while next_group <= req_groups(qt):
                    dma_chunk(next_group)
                    for u in proj_units(next_group):
                        u()
                    next_group += 1
                # queue the next window's groups as pop-filler
                need = (req_groups(qt_order[wi + 1])
                        if wi + 1 < len(qt_order) else -1)
                while next_group <= need:
                    dma_chunk(next_group)
                    proj_q.extend(proj_units(next_group))
                    next_group += 1
                if wi == 1:
                    nc.sync.dma_start(
                        wo_sb[:].rearrange("p (eb o) -> p eb o", eb=2), wo_v[:])
                allow_out[0] = wi == 3
                keep_back[0] = 4 if wi == 3 else 0
                attention(qt)
                while proj_q:  # drain before the next attention tile
                    proj_q.popleft()()
                out_q.extend(partial(outproj_unit, tb, tail=(qt == n_qt - 1))
                             for tb in range(4 * qt, 4 * qt + 4))
            while norm_q:
                norm_q.popleft()()
            while out_q:
                out_q.popleft()()

    nc.compile()
    return nc


def get_compiled(mode: str = "causal") -> "bacc.Bacc":
    nc = _compiled_cache.get(mode)
    if nc is None:
        nc = build_nc(mode)
        _compiled_cache[mode] = nc
    return nc


def _sel_np():
    sel = np.zeros((33, 128), np.float32)
    sel[0, 0:64] = 1.0
    sel[32, 64:128] = 1.0
    return sel


def kernel(query, key, value, mask, Wq, bq, Wk, bk, Wv, bv, Wo, bo):
    query = np.asarray(query, np.float32)
    key = np.asarray(key, np.float32)
    value = np.asarray(value, np.float32)
    mask = np.asarray(mask)
    Wq, bq = np.asarray(Wq, np.float32), np.asarray(bq, np.float32)
    Wk, bk = np.asarray(Wk, np.float32), np.asarray(bk, np.float32)
    Wv, bv = np.asarray(Wv, np.float32), np.asarray(bv, np.float32)
    Wo, bo = np.asarray(Wo, np.float32), np.asarray(bo, np.float32)

    trilm = np.tril(np.ones((S, S), mask.dtype))
    if all(np.array_equal(mask[b], trilm) for b in range(B)):
        mode = "causal"
    elif mask.all():
        mode = "full"
    else:
        raise NotImplementedError("general mask not supported")

    bf = ml_dtypes.bfloat16
    xT = {}
    for nm, arr in (("q", query), ("k", key), ("v", value)):
        xT[nm] = [np.ascontiguousarray(arr[b].T).astype(bf) for b in range(B)]
    WqT = Wq.T.astype(bf)
    WkT = Wk.T.astype(bf)
    WvT = Wv.T.astype(bf)
    WoT = np.ascontiguousarray(Wo.T).astype(bf)
    tri_np = np.where(
        np.arange(128)[:, None] <= np.arange(128)[None, :], 1.0, 0.0
    ).astype(bf)

    in_maps = []
    for c in range(NCORES):
        b, hb = c // 4, c % 4
        es = hb * EPC
        in_maps.append({
            "xq_t": xT["q"][b],
            "xk_t": xT["k"][b],
            "xv_t": xT["v"][b],
            "wq_t": np.ascontiguousarray(WqT[:, es:es + EPC]),
            "wk_t": np.ascontiguousarray(WkT[:, es:es + EPC]),
            "wv_t": np.ascontiguousarray(WvT[:, es:es + EPC]),
            "wo_t": np.ascontiguousarray(WoT[es:es + EPC, :]),
            "bq2": bq[es:es + EPC].reshape(2, 128, 1).astype(np.float32),
            "bk2": bk[es:es + EPC].reshape(2, 128, 1).astype(np.float32),
            "tri": tri_np,
            "ones": _sel_np(),
        })

    nc = get_compiled(mode)
    res = bass_utils.run_bass_kernel_spmd(nc, in_maps, core_ids=list(range(NCORES)))

    const = Wo @ bv + bo
    outf = np.zeros((B, S, D), np.float32)
    for c in range(NCORES):
        outf[c // 4] += res.results[c]["out"].astype(np.float32)
    outf += const[None, None, :]
    return outf
